# revision 2
# baseline (speedup 1.0000x reference)
"""Multi-head attention forward (B=2, S=2048, D=1024, H=16) on 8 TRN2 cores.

Sharding: cores 0-3 take batch 0, cores 4-7 batch 1; each core owns 4 heads
(2 head-pairs of 128 features). Per-core dataflow:

  qT/kT = W @ X.T (+bias)      (PE; DVE tensor_scalar adds bias + evacuates)
  v     = X @ Wv.T directly in [token, feature] layout (lhsT = X.T tiles)
  sT    = kT.T @ qT            (two heads row-tiled, concurrent on PE)
  eT    = exp(sT)              (split ACT / DVE int16-Schraudolph)
  ctxT  = vaug.T @ eT          (65-col lhsT; row 64 = softmax denominators)
  ctx  /= denom                (reciprocal_approx_fast + sel2 broadcast matmul)
  out   = ctxT.T @ WoT         (accumulated over head pairs)

Scheduling: X.T tiles are DMA'd in 4 column strips and cached in SBUF; the
startup issues k0 / v / q0-chunk0 chunk-major so PE consumption tracks DMA
strip arrival. Remaining projections (q0 tail, q1, k1) are closures popped
inside attention(pair0); norm finishes and out-projections pop inside
attention(pair1). kt iterations are processed in pairs to halve PE
weight-geometry switches between the scores and PV matmul shapes.
"""

import sys
import types

import numpy as np

B = 2
S = 2048
D = 1024
H = 16
DK = D // H  # 64
NCORES = 8
CPB = NCORES // B  # 4
FH = D // CPB  # 256
P = 128
KD = D // P  # 8
KT = S // P  # 16
NM = FH // P  # 2 head-pairs per core
QS = 512
NQS = S // QS  # 4
NEG_SCALE = 1.0 / np.sqrt(DK)

# Schraudolph exp via int16 bits of fp16: bits = round(s*EXPA + EXPB),
# reinterpret as fp16 ~= exp(s). Max rel err ~3.1%.
EXPA = float(1024.0 / np.log(2.0))
EXPB = float(15.0 * 1024.0 - 44.0)
DVE_EXP_KTS = ()  # kt tiles whose exp runs on DVE instead of ACT


def _install_ntff_hook():
    if "antenv.axon_hooks" in sys.modules:
        return
    try:
        import antenv
    except ImportError:
        return
    mod = types.ModuleType("antenv.axon_hooks")
    mod._hook = None
    mod.set_axon_ntff_profile_hook = lambda h: setattr(mod, "_hook", h)
    mod.get_axon_ntff_profile_hook = lambda: mod._hook
    sys.modules["antenv.axon_hooks"] = mod
    antenv.axon_hooks = mod
    try:
        from trn_agent_boot.trn_boot import _ntff_profile_via_ctypes

        mod.set_axon_ntff_profile_hook(
            _ntff_profile_via_ctypes("/opt/axon/libaxon_pjrt.so")
        )
    except Exception:
        pass


_NC_CACHE = {}


def _build_nc(debug=False):
    from contextlib import ExitStack

    import concourse.bass as bass  # noqa: F401
    import concourse.mybir as mybir
    import concourse.tile as tile
    from concourse import bacc

    f32 = mybir.dt.float32
    f16 = mybir.dt.float16
    i16 = mybir.dt.int16
    AF = mybir.ActivationFunctionType
    OP = mybir.AluOpType

    nc = bacc.Bacc()

    xtq = nc.dram_tensor("xtq", [D, S], f16, kind="ExternalInput")
    xtk = nc.dram_tensor("xtk", [D, S], f16, kind="ExternalInput")
    xtv = nc.dram_tensor("xtv", [D, S], f16, kind="ExternalInput")
    wqt = nc.dram_tensor("wqt", [D, FH], f16, kind="ExternalInput")
    wkt = nc.dram_tensor("wkt", [D, FH], f16, kind="ExternalInput")
    wvt = nc.dram_tensor("wvt", [D, FH], f16, kind="ExternalInput")
    wot = nc.dram_tensor("wot", [FH, D], f16, kind="ExternalInput")
    bqd = nc.dram_tensor("bqd", [P, NM], f32, kind="ExternalInput")
    bkd = nc.dram_tensor("bkd", [P, NM], f32, kind="ExternalInput")
    bvr = nc.dram_tensor("bvr", [1, FH], f32, kind="ExternalInput")
    sel2d = nc.dram_tensor("sel2d", [2, P], f16, kind="ExternalInput")
    out = nc.dram_tensor("out", [S, D], f16, kind="ExternalOutput")
    if debug:
        dbg_qt = nc.dram_tensor("dbg_qt", [P, NM, S], f16, kind="ExternalOutput")
        dbg_kt = nc.dram_tensor("dbg_kt", [P, NM, S], f16, kind="ExternalOutput")
        dbg_va = nc.dram_tensor("dbg_va", [P, KT, 4 * P], f16, kind="ExternalOutput")
        dbg_cx = nc.dram_tensor("dbg_cx", [P, NM, S], f16, kind="ExternalOutput")
        dbg_ex = nc.dram_tensor("dbg_ex", [P, KT, 2 * QS], f16, kind="ExternalOutput")

    with tile.TileContext(nc) as tc, ExitStack() as ctx:
        const = ctx.enter_context(tc.tile_pool(name="const", bufs=1))
        wpool = ctx.enter_context(tc.tile_pool(name="wpool", bufs=1))
        persist = ctx.enter_context(tc.tile_pool(name="persist", bufs=1))
        xpool = ctx.enter_context(tc.tile_pool(name="xpool", bufs=1))
        expool = ctx.enter_context(tc.tile_pool(name="expool", bufs=7))
        npool = ctx.enter_context(tc.tile_pool(name="npool", bufs=2))
        obpool = ctx.enter_context(tc.tile_pool(name="obpool", bufs=3))
        ps_sc = ctx.enter_context(tc.tile_pool(name="ps_sc", bufs=2, space="PSUM"))
        ps_cx = ctx.enter_context(tc.tile_pool(name="ps_cx", bufs=2, space="PSUM"))
        ps_pp = ctx.enter_context(tc.tile_pool(name="ps_pp", bufs=2, space="PSUM"))

        # --- constants ---
        sel2 = const.tile([2, P], f16)
        nc.sync.dma_start(sel2, sel2d[:, :])
        ones16 = const.tile([P, DK], f16)
        nc.vector.memset(ones16, 1.0)
        ones_row = const.tile([1, P], f32)
        nc.vector.memset(ones_row, 1.0)
        bq_sb = const.tile([P, NM], f32)
        bk_sb = const.tile([P, NM], f32)
        bvr_sb = const.tile([1, FH], f32)
        nc.sync.dma_start(bq_sb, bqd[:, :])
        nc.sync.dma_start(bk_sb, bkd[:, :])
        nc.sync.dma_start(bvr_sb, bvr[:, :])
        # ACT exp-table prewarm (table load overlaps the projection phase)
        warm = const.tile([P, 8], f32)
        warm_o = const.tile([P, 8], f16)
        nc.vector.memset(warm, 0.0)
        nc.scalar.activation(warm_o, warm, AF.Exp)

        # --- weights ---
        wq_sb = wpool.tile([P, KD, FH], f16)
        wk_sb = wpool.tile([P, KD, FH], f16)
        wv_sb = wpool.tile([P, KD, FH], f16)
        wo_sb = wpool.tile([P, NM, D], f16)
        # weight DMAs striped per k-tile; issue order prioritizes what the
        # startup consumes first (wq + xtq strip0, wk + xtk strip0, ...)
        def dma_w(w_sb, wdram, ko):
            nc.sync.dma_start(w_sb[:, ko, :], wdram[ko * P : (ko + 1) * P, :])

        # v-bias broadcast across partitions: bb[p, f] = bv[f]
        bb_ps = ps_pp.tile([P, FH], f32, tag="pp", name="bb_ps")
        nc.tensor.matmul(bb_ps, lhsT=ones_row, rhs=bvr_sb, start=True, stop=True)
        bb_sb = const.tile([P, FH], f32)
        nc.vector.tensor_copy(bb_sb, bb_ps)

        # --- persistent activations ---
        qt_sb = persist.tile([P, NM, S], f16)
        kt_sb = persist.tile([P, NM, S], f16)
        vaug_sb = persist.tile([P, KT, 4 * P], f16)
        ctx_sb = persist.tile([P, NM, S], f16)
        vaug4 = vaug_sb.rearrange("p t (h x) -> p t h x", x=P)
        nc.vector.memset(vaug_sb, 0.0)
        nc.vector.tensor_copy(
            vaug4[:, :, :, DK : DK + 1],
            ones16.rearrange("p (t h x) -> p t h x", h=4, x=1),
        )

        # --- X.T tiles cached in SBUF; DMA'd in 4 column strips, ordered by
        # when the startup consumes them ---
        xt_tiles = {}
        for which, xdram in (("q", xtq), ("k", xtk), ("v", xtv)):
            for ko in range(KD):
                t_ = xpool.tile(
                    [P, S], f16, tag=f"xt_{which}{ko}", name=f"xt_{which}{ko}"
                )
                xt_tiles[(which, ko)] = t_

        def dma_x(which, xdram, ko, strip):
            sl = slice(strip * QS, (strip + 1) * QS)
            nc.sync.dma_start(
                xt_tiles[(which, ko)][:, sl], xdram[ko * P : (ko + 1) * P, sl]
            )

        for ko in range(KD):
            dma_w(wq_sb, wqt, ko)
            dma_x("q", xtq, ko, 0)
        for ko in range(KD):
            dma_w(wk_sb, wkt, ko)
            dma_x("k", xtk, ko, 0)
        for ko in range(KD):
            dma_w(wv_sb, wvt, ko)
            dma_x("v", xtv, ko, 0)
        for strip in range(1, NQS):
            for ko in range(KD):
                dma_x("k", xtk, ko, strip)
                dma_x("v", xtv, ko, strip)
        for strip in range(1, NQS):
            for ko in range(KD):
                dma_x("q", xtq, ko, strip)
        for m in range(NM):
            nc.sync.dma_start(wo_sb[:, m, :], wot[m * P : (m + 1) * P, :])

        # ------------------------------------------------------------------
        # q/k projections: per (pair, proj, ns-chunk), two half-closures of
        # 4 k-tile matmuls; the second half adds bias + evacuates on DVE.
        # ------------------------------------------------------------------
        proj_ps = {}
        wmap = {"q": (wq_sb, bq_sb), "k": (wk_sb, bk_sb)}

        def proj_chunk(pair, which, ns, half=None):
            if half is None:
                proj_chunk(pair, which, ns, 0)
                proj_chunk(pair, which, ns, 1)
                return
            w_sb, b_sb = wmap[which]
            key = (pair, which, ns)
            if key not in proj_ps:
                proj_ps[key] = ps_pp.tile(
                    [P, QS], f32, tag="pp", name=f"pp_{pair}_{which}_{ns}"
                )
            ps = proj_ps[key]
            for ko in range(4 * half, 4 * half + 4):
                nc.tensor.matmul(
                    ps,
                    lhsT=w_sb[:, ko, pair * P : (pair + 1) * P],
                    rhs=xt_tiles[(which, ko)][:, ns * QS : (ns + 1) * QS],
                    start=(ko == 0),
                    stop=(ko == KD - 1),
                )
            if half == 1:
                dst = (qt_sb if which == "q" else kt_sb)[
                    :, pair, ns * QS : (ns + 1) * QS
                ]
                nc.vector.tensor_scalar(dst, ps, b_sb[:, pair : pair + 1], None, OP.add)

        # v projection directly into [token, feature] layout: one PSUM tile
        # per token-tile st, contracting over ko with X.T tiles as weights.
        def v_chunk(st):
            vp = ps_pp.tile([P, FH], f32, tag="pp", name=f"vp_{st}")
            for ko in range(KD):
                nc.tensor.matmul(
                    vp,
                    lhsT=xt_tiles[("v", ko)][:, st * P : (st + 1) * P],
                    rhs=wv_sb[:, ko, :],
                    start=(ko == 0),
                    stop=(ko == KD - 1),
                )
            nc.vector.tensor_add(
                vaug4[:, st, 0:4, 0:DK],
                vp.rearrange("p (h x) -> p h x", x=DK),
                bb_sb.rearrange("p (h x) -> p h x", x=DK),
            )

        # ------------------------------------------------------------------
        # attention helpers
        # ------------------------------------------------------------------
        def pv_kt(pair, cx, kt, ex):
            for h in range(2):
                nc.tensor.matmul(
                    cx[h][0 : DK + 1, :],
                    lhsT=vaug4[:, kt, 2 * pair + h, 0 : DK + 1],
                    rhs=ex[:, h * QS : (h + 1) * QS],
                    start=(kt == 0),
                    stop=(kt == KT - 1),
                )

        def norm_head(pair, qs, cx):
            cxs = [
                npool.tile(
                    [DK + 1, QS], f32, tag="cxs", name=f"cxs{pair}_{qs}_{h}", bufs=6
                )
                for h in range(2)
            ]
            s2 = npool.tile([2, QS], f32, tag="s2", name=f"s2_{pair}_{qs}", bufs=4)
            for h in range(2):
                nc.vector.tensor_copy(cxs[h], cx[h][0 : DK + 1, :])
                nc.sync.dma_start(s2[h : h + 1, :], cxs[h][DK : DK + 1, :])

            def finish():
                rc2 = npool.tile([2, QS], f32, tag="rc2", name=f"rc2_{pair}_{qs}")
                rch = npool.tile([2, QS], f16, tag="rch", name=f"rch_{pair}_{qs}")
                nc.vector.reciprocal_approx_fast(rc2, s2)
                nc.vector.tensor_copy(rch, rc2)
                bc = ps_pp.tile([P, QS], f32, tag="pp", name=f"bc_{pair}_{qs}")
                nc.tensor.matmul(bc, lhsT=sel2, rhs=rch, start=True, stop=True)
                for h in range(2):
                    nc.vector.tensor_mul(
                        ctx_sb[DK * h : DK * (h + 1), pair, qs * QS : (qs + 1) * QS],
                        cxs[h][0:DK, :],
                        bc[DK * h : DK * (h + 1), :],
                    )

            return finish

        def out_proj_mt(mt):
            for nso in range(2):
                ops = ps_pp.tile([P, QS], f32, tag="pp", name=f"op{mt}_{nso}")
                for pair in range(NM):
                    nc.tensor.matmul(
                        ops,
                        lhsT=ctx_sb[:, pair, mt * P : (mt + 1) * P],
                        rhs=wo_sb[:, pair, nso * QS : (nso + 1) * QS],
                        start=(pair == 0),
                        stop=(pair == NM - 1),
                    )
                ob = obpool.tile([P, QS], f16, tag="ob", name=f"ob{mt}_{nso}")
                # alternate evacuation engine to balance ACT/DVE load
                if nso == 0:
                    nc.scalar.copy(ob, ops)
                else:
                    nc.vector.tensor_copy(ob, ops)
                # two DMA chunks on separate queues to halve the drain tail
                for c in range(2):
                    csl = slice(nso * QS + c * 256, nso * QS + (c + 1) * 256)
                    nc.sync.dma_start(
                        out[mt * P : (mt + 1) * P, csl],
                        ob[:, c * 256 : (c + 1) * 256],
                    )

        # ------------------------------------------------------------------
        # main flow
        # ------------------------------------------------------------------
        # startup, chunk-major so PE work tracks DMA strip arrival:
        # q0 chunk 0 + k0 chunks + first half of the v token-tiles
        proj_chunk(0, "q", 0)
        proj_chunk(0, "k", 0)
        v_chunk(0)
        v_chunk(1)
        for ns in range(1, NQS):
            proj_chunk(0, "k", ns)
            v_chunk(2 * ns)
            v_chunk(2 * ns + 1)

        # remaining projections pop inside attention(pair0), ordered by
        # deadline: v st8-15 feed pair-0 PV in round 0, q0 chunk g feeds
        # round g, k1/q1 feed attention(pair1)
        pend = []

        def mk_proj(pair, which, ns):
            return lambda: proj_chunk(pair, which, ns)

        for st in range(8, 12):
            pend.append(lambda st=st: v_chunk(st))
        pend.append(mk_proj(0, "q", 1))
        for st in range(12, 16):
            pend.append(lambda st=st: v_chunk(st))
        for ns in range(2, NQS):
            pend.append(mk_proj(0, "q", ns))
        for which in ("k", "q"):
            for ns in range(NQS):
                pend.append(mk_proj(1, which, ns))

        def attention(pair):
            for qs in range(NQS):
                cx = [
                    ps_cx.tile([P, QS], f32, tag="cx", name=f"cx{pair}_{qs}_{h}")
                    for h in range(2)
                ]
                exq = []
                for kt2 in range(0, KT, 2):
                    # two kt of scores + exp back-to-back, then two kt of PV:
                    # halves PE weight-geometry switches between the 64-row
                    # scores matmuls and the 128-row PV matmuls.
                    for kt in (kt2, kt2 + 1):
                        sc = ps_sc.tile(
                            [P, 2 * QS], f32, tag="sc", name=f"sc{pair}_{qs}_{kt}"
                        )
                        for h in range(2):
                            nc.tensor.matmul(
                                sc[:, h * QS : (h + 1) * QS],
                                lhsT=kt_sb[
                                    DK * h : DK * (h + 1), pair, kt * P : (kt + 1) * P
                                ],
                                rhs=qt_sb[
                                    DK * h : DK * (h + 1), pair, qs * QS : (qs + 1) * QS
                                ],
                                start=True,
                                stop=True,
                            )
                        ex = expool.tile([P, 2 * QS], f16, tag="ex")
                        if kt in DVE_EXP_KTS:
                            nc.vector.tensor_scalar(
                                ex.bitcast(i16), sc, EXPA, EXPB, OP.mult, OP.add
                            )
                        else:
                            nc.scalar.activation(ex, sc, AF.Exp)
                        if debug and pair == 0 and qs == 0:
                            nc.sync.dma_start(dbg_ex[:, kt, :], ex[:, :])
                        exq.append((kt, ex))
                    if kt2 > 0 and pend:
                        pend.pop(0)()
                    if kt2 in (4, 10) and pend:
                        pend.pop(0)()
                    while len(exq) > 4:
                        k2, e2 = exq.pop(0)
                        pv_kt(pair, cx, k2, e2)
                if pend:
                    pend.pop(0)()
                for k2, e2 in exq:
                    pv_kt(pair, cx, k2, e2)
                fin = norm_head(pair, qs, cx)
                pend.append(fin)
                if pair == 1:
                    for mt in range(qs * 4, qs * 4 + 4):
                        pend.append(lambda mt=mt: out_proj_mt(mt))

        attention(0)
        attention(1)
        for f in pend:
            f()
        if debug:
            nc.sync.dma_start(dbg_qt[:, :, :], qt_sb)
            nc.sync.dma_start(dbg_kt[:, :, :], kt_sb)
            nc.sync.dma_start(dbg_va[:, :, :], vaug_sb)
            nc.sync.dma_start(dbg_cx[:, :, :], ctx_sb)

    nc.finalize()
    return nc


def _get_nc():
    if "nc" not in _NC_CACHE:
        _install_ntff_hook()
        _NC_CACHE["nc"] = _build_nc()
    return _NC_CACHE["nc"]


def _make_in_maps(query, key, value, Wq, bq, Wk, bk, Wv, bv, Wo):
    qn = np.asarray(query, np.float32)
    kn = np.asarray(key, np.float32)
    vn = np.asarray(value, np.float32)
    Wq = np.asarray(Wq, np.float32)
    Wk = np.asarray(Wk, np.float32)
    Wv = np.asarray(Wv, np.float32)
    Wo = np.asarray(Wo, np.float32)
    bq = np.asarray(bq, np.float32)
    bk = np.asarray(bk, np.float32)
    bv = np.asarray(bv, np.float32)

    sel2 = np.zeros((2, P), np.float16)
    sel2[0, 0:DK] = 1.0
    sel2[1, DK : 2 * DK] = 1.0

    xt = {}
    for b in range(B):
        xt[b] = (
            np.ascontiguousarray(qn[b].T).astype(np.float16),
            np.ascontiguousarray(kn[b].T).astype(np.float16),
            np.ascontiguousarray(vn[b].T).astype(np.float16),
        )

    in_maps = []
    for c in range(NCORES):
        b, hp = divmod(c, CPB)
        sl = slice(hp * FH, (hp + 1) * FH)
        in_maps.append(
            {
                "xtq": xt[b][0],
                "xtk": xt[b][1],
                "xtv": xt[b][2],
                "wqt": np.ascontiguousarray((Wq[sl, :] * NEG_SCALE).T).astype(
                    np.float16
                ),
                "wkt": np.ascontiguousarray(Wk[sl, :].T).astype(np.float16),
                "wvt": np.ascontiguousarray(Wv[sl, :].T).astype(np.float16),
                "wot": np.ascontiguousarray(Wo[:, sl].T).astype(np.float16),
                "bqd": np.ascontiguousarray((bq[sl] * NEG_SCALE).reshape(NM, P).T),
                "bkd": np.ascontiguousarray(bk[sl].reshape(NM, P).T),
                "bvr": np.ascontiguousarray(bv[sl].reshape(1, FH)),
                "sel2d": sel2,
            }
        )
    return in_maps


def _run(inputs, trace=False):
    from concourse.bass_utils import run_bass_kernel_spmd

    nc = _get_nc()
    in_maps = _make_in_maps(
        inputs["query"],
        inputs["key"],
        inputs["value"],
        inputs["Wq"],
        inputs["bq"],
        inputs["Wk"],
        inputs["bk"],
        inputs["Wv"],
        inputs["bv"],
        inputs["Wo"],
    )
    res = run_bass_kernel_spmd(nc, in_maps, list(range(NCORES)), trace=trace)
    bo = np.asarray(inputs["bo"], np.float32)
    out = np.zeros((B, S, D), np.float32)
    for c in range(NCORES):
        out[c // CPB] += res.results[c]["out"].astype(np.float32)
    out += bo[None, None, :]
    return out, res


def kernel(**inputs) -> np.ndarray:
    out, _ = _run(inputs, trace=False)
    return out
